# revision 15
# baseline (speedup 1.0000x reference)
"""Trainium2 Bass kernel for DoubleBinaryLinear:
    y = ((x * s0) @ B.T * s2) @ A.T * s4 + bias
with x [4, 2048, 4096] fp32 and binary (+-1) B, A [4096, 4096].

v6b: fused-weight restructure + one-level Strassen on the token pass,
output-sharded across the 8 cores.

    M.T = diag(s0) B.T (diag(s2) A.T)   # [in, out], token-independent
    y   = x @ M.T * s4 + bias           # x pre-cast fp16 on host

Core c computes M.T columns for its 512 output rows (one 4096x4096x512
matmul, ~220 us) and keeps M.T resident in SBUF. The 8192-token x pass
then runs one-level Strassen per 512-token chunk: out 512 -> 2x256,
in 4096 -> 2x2048, tokens 512 -> 2x256, so each chunk does 7/8 of the
plain multiply work (224 matmuls at N=256 ~ 109 ns each vs 128 at
N=512 ~ 216 ns). Operand combines (5 weight sets built once from M.T,
5 x-combos per chunk) run on the Vector engine, which has ~2x headroom
against TensorE. PSUM products recombine on eviction via
scalar_tensor_tensor chains into the final s4*y+bias stores.

a2 = s2*A.T is precomputed on host and shipped fp16. Chunk 0 runs the
plain (non-Strassen) multiply interleaved into phase B to absorb B-DMA
jitter; phase-B in-tile groups run in the order 0,4,1,5,... so the
Strassen weight combos (which pair in-tile j with 16+j) can be built
incrementally during phase B.

Numerics: fp16 single-rounding everywhere, fp32 PSUM; Strassen combos
add ~2x to the fp16 noise (measured 8.8e-4 rel on CPU vs 2e-2 gate).
"""

import os

import numpy as np

import concourse.bacc as bacc
import concourse.mybir as mybir
from concourse import tile
from concourse import bass_utils

P = 128
F32 = mybir.dt.float32
F16 = mybir.dt.float16

IN_D = 4096
MID_D = 4096
OUT_D = 4096
BATCH = 4
SEQ = 2048
N_CORES = 8
T_ALL = BATCH * SEQ                 # 8192 tokens, every core sees all
OS = OUT_D // N_CORES               # 512 output rows per core
TC = 512                            # token chunk
HC = TC // 2                        # Strassen token half-chunk (256)
nI = IN_D // P                      # 32 in tiles
nM = MID_D // P                     # 32 mid tiles
nJ = nI // 2                        # 16 in-tile pairs (j, 16+j)
nOB = OS // P                       # 4 out blocks per core
nTC = T_ALL // TC                   # 16 token chunks
IG = 4                              # in-tiles per M-compute PSUM group

mult = mybir.AluOpType.mult
add = mybir.AluOpType.add

# phase-B group order: pair-completing so Strassen weight combos
# (needing in-tiles j and 16+j) can be built during phase B.
GORDER = [0, 4, 1, 5, 2, 6, 3, 7]


def _build_nc():
    nc = bacc.Bacc(None, target_bir_lowering=False)
    xTd = nc.dram_tensor("xT", [IN_D, T_ALL], F16, kind="ExternalInput")
    Bd = nc.dram_tensor("B", [MID_D, IN_D], F16, kind="ExternalInput")
    a2d = nc.dram_tensor("a2", [MID_D, OS], F16, kind="ExternalInput")
    nSC = nI + 2 * nOB
    scd = nc.dram_tensor("sc", [P, nSC], F32, kind="ExternalInput")
    yTd = nc.dram_tensor("yT", [OS, T_ALL], F32, kind="ExternalOutput")

    with tile.TileContext(nc) as tc:
        with (
            tc.tile_pool(name="consts", bufs=1) as cpool,
            tc.tile_pool(name="mtbuf", bufs=1) as mpool,
            tc.tile_pool(name="xin", bufs=2) as xpool,
            tc.tile_pool(name="yout", bufs=4) as ypool,
            tc.tile_pool(name="wcmb", bufs=1) as wpool,
            tc.tile_pool(name="psum", bufs=8, space="PSUM") as pspool,
        ):
            def tt(out, in0, in1, op, eng=None):
                (eng or nc.vector).tensor_tensor(out, in0, in1, op)

            sc_t = cpool.tile([P, nSC], F32, tag="sc")
            nc.scalar.dma_start(sc_t[:], scd[:, :])
            s0_t = sc_t[:, 0:nI]
            s4_t = sc_t[:, nI:nI + nOB]
            bi_t = sc_t[:, nI + nOB:nSC]

            # Strassen weight combos [128, 256] fp16, built from mt during
            # phase B. w[k][j] is the stationary operand set for product k.
            wc = {k: [None] * nJ for k in (1, 2, 5, 6, 7)}

            mt = [mpool.tile([P, OS], F16, tag=f"m{it}", name=f"m{it}")
                  for it in range(nI)]

            sub = mybir.AluOpType.subtract

            def build_wcombos(js):
                # Stationary-layout blocks (W[I,O] slices of mt):
                #   A11 -> mt[j][:, 0:256]      A12 -> mt[16+j][:, 0:256]
                #   A21 -> mt[j][:, 256:512]    A22 -> mt[16+j][:, 256:512]
                # Product-k stationary combos:
                #   w1 = A11+A22, w2 = A21+A22, w5 = A11+A12,
                #   w6 = A21-A11, w7 = A12-A22
                H2 = 2 * P
                for j in js:
                    lo, hi = mt[j], mt[nJ + j]
                    specs = {
                        1: (lo[:, 0:H2], hi[:, H2:2 * H2], add),
                        2: (lo[:, H2:2 * H2], hi[:, H2:2 * H2], add),
                        5: (lo[:, 0:H2], hi[:, 0:H2], add),
                        6: (lo[:, H2:2 * H2], lo[:, 0:H2], sub),
                        7: (hi[:, 0:H2], hi[:, H2:2 * H2], sub),
                    }
                    for k, (in0, in1, op) in specs.items():
                        w = wpool.tile([P, H2], F16, tag=f"w{k}_{j}",
                                       name=f"w{k}_{j}")
                        tt(w[:], in0, in1, op, eng=nc.gpsimd)
                        wc[k][j] = w

            with (
                tc.tile_pool(name="a2buf", bufs=1) as apool,
                tc.tile_pool(name="bwts", bufs=16) as bpool,
            ):
                # a2 tiles [128 mid, OS], precomputed on host.
                a2 = []
                for mk in range(nM):
                    a2t = apool.tile([P, OS], F16, tag=f"a{mk}", name=f"a{mk}")
                    nc.scalar.dma_start(a2t[:], a2d[mk * P:(mk + 1) * P, :])
                    a2.append(a2t)

                # Chunk-0 x tiles prefetch behind the a2 loads; chunk-0 runs
                # plain (non-Strassen), interleaved into phase B.
                x0 = []
                for it in range(nI):
                    xt = xpool.tile([P, TC], F16, tag=f"x{it}", name=f"x{it}")
                    nc.scalar.dma_start(xt[:], xTd[it * P:(it + 1) * P, 0:TC])
                    x0.append(xt)
                ps0 = [pspool.tile([P, TC], F32, tag="ps0", name="ps0", bufs=4)
                       for _ in range(nOB)]

                x0_cnt = [0] * nOB

                def x0_mm(slot, git):
                    # one chunk-0 matmul for in-tile group git, slot 0..15
                    it = git * IG + slot // nOB
                    ob = slot % nOB
                    c = x0_cnt[ob]
                    x0_cnt[ob] += 1
                    nc.tensor.matmul(ps0[ob][:],
                                     mt[it][:, ob * P:(ob + 1) * P],
                                     x0[it][:], start=(c == 0),
                                     stop=(c == nI - 1))

                # phase B: M.T tiles [128 in, OS]; s0 folded into eviction.
                for gi, ig in enumerate(GORDER):
                    psb = [pspool.tile([P, OS], F32, tag="ps", name="ps",
                                       bufs=4) for _ in range(IG)]
                    for mk in range(nM):
                        bt = bpool.tile([P, IG * P], F16, tag="wb")
                        nc.sync.dma_start(
                            bt[:], Bd[mk * P:(mk + 1) * P,
                                      ig * IG * P:(ig + 1) * IG * P])
                        last = mk == nM - 1
                        for t_ in range(IG):
                            nc.tensor.matmul(psb[t_][:],
                                             bt[:, t_ * P:(t_ + 1) * P],
                                             a2[mk][:], start=(mk == 0),
                                             stop=last)
                        if gi > 0 and mk % 2 == 1:
                            x0_mm(mk // 2, GORDER[gi - 1])
                    for t_ in range(IG):
                        it = ig * IG + t_
                        nc.vector.tensor_scalar_mul(mt[it][:], psb[t_][:],
                                                    s0_t[:, it:it + 1])
                    if gi % 2 == 1:
                        # pair {GORDER[gi-1], ig} complete: in-tile pairs
                        # j = 4*GORDER[gi-1] .. +3 are ready.
                        g0 = GORDER[gi - 1]
                        build_wcombos(range(4 * g0, 4 * g0 + 4))
                for slot in range(16):
                    x0_mm(slot, GORDER[-1])
                for ob in range(nOB):
                    for h in range(2):
                        yt = ypool.tile([P, HC], F32, tag="yt")
                        nc.vector.tensor_scalar(
                            yt[:], ps0[ob][:, h * HC:(h + 1) * HC],
                            s4_t[:, ob:ob + 1], bi_t[:, ob:ob + 1], mult, add)
                        nc.sync.dma_start(
                            yTd[ob * P:(ob + 1) * P, h * HC:(h + 1) * HC],
                            yt[:])

            # phase C: Strassen chunks 1..15.
            with (
                tc.tile_pool(name="ucmb", bufs=4) as upool,
                tc.tile_pool(name="ctmp", bufs=2) as ctpool,
            ):
                def uc(k, j, i0, h0, i1, h1, op, xts, eng=None):
                    # x-combo tile = xts[i0][:, h0-half] op xts[i1][:, h1-half]
                    u = upool.tile([P, HC], F16, tag=f"u{k}", name=f"u{k}")
                    tt(u[:], xts[i0][:, h0 * HC:(h0 + 1) * HC],
                       xts[i1][:, h1 * HC:(h1 + 1) * HC], op, eng=eng)
                    return u

                def evict(t, blk, th, t0):
                    # t [128, 512] fp32 (ob2 halves side by side) ->
                    # y rows blk*128+[ob2], tokens [t0+th*256, +256)
                    for ob2 in range(2):
                        ob = blk * 2 + ob2
                        yt = ypool.tile([P, HC], F32, tag="yt")
                        nc.vector.tensor_scalar(
                            yt[:], t[:, ob2 * HC:(ob2 + 1) * HC],
                            s4_t[:, ob:ob + 1], bi_t[:, ob:ob + 1], mult, add)
                        nc.sync.dma_start(
                            yTd[ob * P:(ob + 1) * P,
                                t0 + th * HC:t0 + (th + 1) * HC], yt[:])

                for c in range(1, nTC):
                    t0 = c * TC
                    xts = []
                    for it in range(nI):
                        xt = xpool.tile([P, TC], F16, tag=f"x{it}",
                                        name=f"x{it}")
                        nc.scalar.dma_start(
                            xt[:], xTd[it * P:(it + 1) * P, t0:t0 + TC])
                        xts.append(xt)

                    # psum products [128, 512]: ob2 halves side by side.
                    pm = {}
                    for k, tg in ((1, "ps"), (2, "ps"), (3, "ps"), (4, "ps"),
                                  (6, "ps0"), (5, "ps0"), (7, "ps0")):
                        pm[k] = pspool.tile([P, TC], F32, tag=tg,
                                            name=f"pm{k}", bufs=4)

                    def prod(k, wset, mov):
                        # wset: list of 16 stationary [128, 256] tiles (or
                        # slices); mov: list of 16 moving [128, 256] APs.
                        # start=True clears the whole PSUM bank, so only the
                        # very first matmul into the tile may carry it; the
                        # second ob2 region then overwrites zeroed memory
                        # (has_written=0) which is equivalent to a start.
                        for j in range(nJ):
                            for ob2 in range(2):
                                nc.tensor.matmul(
                                    pm[k][:, ob2 * HC:(ob2 + 1) * HC],
                                    wset[j][:, ob2 * P:(ob2 + 1) * P],
                                    mov[j], start=(j == 0 and ob2 == 0),
                                    stop=(j == nJ - 1),
                                    skip_group_check=True)

                    # moving operands
                    xa = lambda i: [xts[i + j][:, 0:HC] for j in range(nJ)]
                    xb = lambda i: [xts[i + j][:, HC:TC] for j in range(nJ)]

                    # M1 = (A11+A22)(B11+B22)
                    u1 = [uc(1, j, j, 0, nJ + j, 1, add, xts)
                          for j in range(nJ)]
                    prod(1, wc[1], [u[:] for u in u1])
                    # M2 = (A21+A22) B11   (raw, strided moving operand)
                    prod(2, wc[2], xa(0))
                    # M3 = A11 (B12-B22)
                    u3 = [uc(3, j, j, 1, nJ + j, 1, sub, xts)
                          for j in range(nJ)]
                    prod(3, [mt[j][:, 0:2 * P] for j in range(nJ)],
                         [u[:] for u in u3])
                    t22 = ctpool.tile([P, TC], F32, tag="t22")
                    nc.vector.tensor_scalar_add(t22[:], pm[1][:], 0.0)
                    tt(t22[:], t22[:], pm[2][:], sub)               # M1-M2
                    # M6 = (A21-A11)(B11+B12)
                    u6 = [uc(6, j, j, 0, j, 1, add, xts, eng=nc.gpsimd)
                          for j in range(nJ)]
                    prod(6, wc[6], [u[:] for u in u6])
                    tt(t22[:], t22[:], pm[3][:], add)               # +M3
                    tt(t22[:], t22[:], pm[6][:], add)               # +M6
                    evict(t22, 1, 1, t0)                            # C22
                    # M4 = A22 (B21-B11)
                    u4 = [uc(4, j, nJ + j, 0, j, 0, sub, xts, eng=nc.gpsimd)
                          for j in range(nJ)]
                    prod(4, [mt[nJ + j][:, 2 * P:4 * P] for j in range(nJ)],
                         [u[:] for u in u4])
                    t21 = ctpool.tile([P, TC], F32, tag="t21")
                    nc.vector.tensor_scalar_add(t21[:], pm[2][:], 0.0)
                    tt(t21[:], t21[:], pm[4][:], add)               # M2+M4
                    evict(t21, 1, 0, t0)                            # C21
                    t11 = ctpool.tile([P, TC], F32, tag="t11")
                    nc.vector.tensor_scalar_add(t11[:], pm[1][:], 0.0)
                    tt(t11[:], t11[:], pm[4][:], add)               # M1+M4
                    # M5 = (A11+A12) B22   (raw, strided moving operand)
                    prod(5, wc[5], xb(nJ))
                    t12 = ctpool.tile([P, TC], F32, tag="t12")
                    nc.vector.tensor_scalar_add(t12[:], pm[3][:], 0.0)
                    tt(t12[:], t12[:], pm[5][:], add)               # M3+M5
                    evict(t12, 0, 1, t0)                            # C12
                    tt(t11[:], t11[:], pm[5][:], sub)               # -M5
                    # M7 = (A12-A22)(B21+B22)
                    u7 = [uc(7, j, nJ + j, 0, nJ + j, 1, add, xts,
                             eng=nc.gpsimd) for j in range(nJ)]
                    prod(7, wc[7], [u[:] for u in u7])
                    tt(t11[:], t11[:], pm[7][:], add)               # +M7
                    evict(t11, 0, 0, t0)                            # C11

    nc.compile()
    return nc


_NC_CACHE = None


def _get_nc():
    global _NC_CACHE
    if _NC_CACHE is None:
        _NC_CACHE = _build_nc()
    return _NC_CACHE


def _col_major(v):
    return np.ascontiguousarray(
        np.asarray(v, dtype=np.float32).reshape(-1, P).T)


def make_in_maps(x, scaling0, B, scaling2, A, scaling4, bias):
    xh = np.asarray(x, dtype=np.float32).reshape(T_ALL, IN_D).astype(np.float16)
    xT = np.ascontiguousarray(xh.T)
    B16 = np.asarray(B, dtype=np.float32).astype(np.float16)
    a2_full = (np.asarray(scaling2, dtype=np.float32)[:, None]
               * np.asarray(A, dtype=np.float32).T).astype(np.float16)
    s0c = _col_major(scaling0)

    in_maps = []
    for c in range(N_CORES):
        sh = slice(c * OS, (c + 1) * OS)
        sc = np.ascontiguousarray(np.concatenate(
            [s0c, _col_major(np.asarray(scaling4)[sh]),
             _col_major(np.asarray(bias)[sh])], axis=1))
        in_maps.append({
            "xT": xT, "B": B16,
            "a2": np.ascontiguousarray(a2_full[:, sh]),
            "sc": sc,
        })
    return in_maps


def _unshard(results):
    y = np.empty((T_ALL, OUT_D), dtype=np.float32)
    for c in range(N_CORES):
        y[:, c * OS:(c + 1) * OS] = results[c]["yT"].T
    return y.reshape(BATCH, SEQ, OUT_D)


def kernel(x, scaling0, B, scaling2, A, scaling4, bias):
    # The profile hook isn't available in every environment; force the
    # plain execution path.
    os.environ.setdefault("BASS_NEVER_TRACE", "1")

    in_maps = make_in_maps(x, scaling0, B, scaling2, A, scaling4, bias)
    nc = _get_nc()
    res = bass_utils.run_bass_kernel_spmd(
        nc, in_maps, core_ids=list(range(N_CORES)))
    return _unshard(res.results)


# revision 16
# speedup vs baseline: 1.1140x; 1.1140x over previous
"""Trainium2 Bass kernel for DoubleBinaryLinear:
    y = ((x * s0) @ B.T * s2) @ A.T * s4 + bias
with x [4, 2048, 4096] fp32 and binary (+-1) B, A [4096, 4096].

v6b: fused-weight restructure + one-level Strassen on the token pass,
output-sharded across the 8 cores.

    M.T = diag(s0) B.T (diag(s2) A.T)   # [in, out], token-independent
    y   = x @ M.T * s4 + bias           # x pre-cast fp16 on host

Core c computes M.T columns for its 512 output rows (one 4096x4096x512
matmul, ~220 us) and keeps M.T resident in SBUF. The 8192-token x pass
then runs one-level Strassen per 512-token chunk: out 512 -> 2x256,
in 4096 -> 2x2048, tokens 512 -> 2x256, so each chunk does 7/8 of the
plain multiply work (224 matmuls at N=256 ~ 109 ns each vs 128 at
N=512 ~ 216 ns). Operand combines (5 weight sets built once from M.T,
5 x-combos per chunk) run on the Vector engine, which has ~2x headroom
against TensorE. PSUM products recombine on eviction via
scalar_tensor_tensor chains into the final s4*y+bias stores.

a2 = s2*A.T is precomputed on host and shipped fp16. Chunk 0 runs the
plain (non-Strassen) multiply interleaved into phase B to absorb B-DMA
jitter; phase-B in-tile groups run in the order 0,4,1,5,... so the
Strassen weight combos (which pair in-tile j with 16+j) can be built
incrementally during phase B.

Numerics: fp16 single-rounding everywhere, fp32 PSUM; Strassen combos
add ~2x to the fp16 noise (measured 8.8e-4 rel on CPU vs 2e-2 gate).
"""

import os

import numpy as np

import concourse.bacc as bacc
import concourse.mybir as mybir
from concourse import tile
from concourse import bass_utils

P = 128
F32 = mybir.dt.float32
F16 = mybir.dt.float16

IN_D = 4096
MID_D = 4096
OUT_D = 4096
BATCH = 4
SEQ = 2048
N_CORES = 8
T_ALL = BATCH * SEQ                 # 8192 tokens, every core sees all
OS = OUT_D // N_CORES               # 512 output rows per core
TC = 512                            # token chunk
HC = TC // 2                        # Strassen token half-chunk (256)
nI = IN_D // P                      # 32 in tiles
nM = MID_D // P                     # 32 mid tiles
nJ = nI // 2                        # 16 in-tile pairs (j, 16+j)
nOB = OS // P                       # 4 out blocks per core
nTC = T_ALL // TC                   # 16 token chunks
IG = 4                              # in-tiles per M-compute PSUM group

mult = mybir.AluOpType.mult
add = mybir.AluOpType.add
CPY = mybir.ActivationFunctionType.Copy

# phase-B group order: pair-completing so Strassen weight combos
# (needing in-tiles j and 16+j) can be built during phase B.
GORDER = [0, 4, 1, 5, 2, 6, 3, 7]


def _build_nc():
    nc = bacc.Bacc(None, target_bir_lowering=False)
    xTd = nc.dram_tensor("xT", [IN_D, T_ALL], F16, kind="ExternalInput")
    Bd = nc.dram_tensor("B", [MID_D, IN_D], F16, kind="ExternalInput")
    a2d = nc.dram_tensor("a2", [MID_D, OS], F16, kind="ExternalInput")
    nSC = nI + 2 * nOB
    scd = nc.dram_tensor("sc", [P, nSC], F32, kind="ExternalInput")
    yTd = nc.dram_tensor("yT", [OS, T_ALL], F32, kind="ExternalOutput")

    with tile.TileContext(nc) as tc:
        with (
            tc.tile_pool(name="consts", bufs=1) as cpool,
            tc.tile_pool(name="mtbuf", bufs=1) as mpool,
            tc.tile_pool(name="xin", bufs=2) as xpool,
            tc.tile_pool(name="yout", bufs=4) as ypool,
            tc.tile_pool(name="wcmb", bufs=1) as wpool,
            tc.tile_pool(name="psum", bufs=8, space="PSUM") as pspool,
        ):
            def tt(out, in0, in1, op, eng=None):
                (eng or nc.vector).tensor_tensor(out, in0, in1, op)

            sc_t = cpool.tile([P, nSC], F32, tag="sc")
            nc.scalar.dma_start(sc_t[:], scd[:, :])
            s0_t = sc_t[:, 0:nI]
            s4_t = sc_t[:, nI:nI + nOB]
            bi_t = sc_t[:, nI + nOB:nSC]

            # Strassen weight combos [128, 256] fp16, built from mt during
            # phase B. w[k][j] is the stationary operand set for product k.
            wc = {k: [None] * nJ for k in (1, 2, 5, 6, 7)}

            mt = [mpool.tile([P, OS], F16, tag=f"m{it}", name=f"m{it}")
                  for it in range(nI)]

            sub = mybir.AluOpType.subtract

            def build_wcombos(js):
                # Stationary-layout blocks (W[I,O] slices of mt):
                #   A11 -> mt[j][:, 0:256]      A12 -> mt[16+j][:, 0:256]
                #   A21 -> mt[j][:, 256:512]    A22 -> mt[16+j][:, 256:512]
                # Product-k stationary combos:
                #   w1 = A11+A22, w2 = A21+A22, w5 = A11+A12,
                #   w6 = A21-A11, w7 = A12-A22
                H2 = 2 * P
                for j in js:
                    lo, hi = mt[j], mt[nJ + j]
                    specs = {
                        1: (lo[:, 0:H2], hi[:, H2:2 * H2], add),
                        2: (lo[:, H2:2 * H2], hi[:, H2:2 * H2], add),
                        5: (lo[:, 0:H2], hi[:, 0:H2], add),
                        6: (lo[:, H2:2 * H2], lo[:, 0:H2], sub),
                        7: (hi[:, 0:H2], hi[:, H2:2 * H2], sub),
                    }
                    for k, (in0, in1, op) in specs.items():
                        w = wpool.tile([P, H2], F16, tag=f"w{k}_{j}",
                                       name=f"w{k}_{j}")
                        tt(w[:], in0, in1, op, eng=nc.gpsimd)
                        wc[k][j] = w

            with (
                tc.tile_pool(name="a2buf", bufs=1) as apool,
                tc.tile_pool(name="bwts", bufs=16) as bpool,
            ):
                # a2 tiles [128 mid, OS], precomputed on host.
                a2 = []
                for mk in range(nM):
                    a2t = apool.tile([P, OS], F16, tag=f"a{mk}", name=f"a{mk}")
                    nc.scalar.dma_start(a2t[:], a2d[mk * P:(mk + 1) * P, :])
                    a2.append(a2t)

                # Chunk-0 x tiles prefetch behind the a2 loads; chunk-0 runs
                # plain (non-Strassen), interleaved into phase B.
                x0 = []
                for it in range(nI):
                    xt = xpool.tile([P, TC], F16, tag=f"x{it}", name=f"x{it}")
                    nc.scalar.dma_start(xt[:], xTd[it * P:(it + 1) * P, 0:TC])
                    x0.append(xt)
                ps0 = [pspool.tile([P, TC], F32, tag="ps0", name="ps0", bufs=4)
                       for _ in range(nOB)]

                x0_cnt = [0] * nOB

                def x0_mm(slot, git):
                    # one chunk-0 matmul for in-tile group git, slot 0..15
                    it = git * IG + slot // nOB
                    ob = slot % nOB
                    c = x0_cnt[ob]
                    x0_cnt[ob] += 1
                    nc.tensor.matmul(ps0[ob][:],
                                     mt[it][:, ob * P:(ob + 1) * P],
                                     x0[it][:], start=(c == 0),
                                     stop=(c == nI - 1))

                # phase B: M.T tiles [128 in, OS]; s0 folded into eviction.
                for gi, ig in enumerate(GORDER):
                    psb = [pspool.tile([P, OS], F32, tag="ps", name="ps",
                                       bufs=4) for _ in range(IG)]
                    for mk in range(nM):
                        bt = bpool.tile([P, IG * P], F16, tag="wb")
                        nc.sync.dma_start(
                            bt[:], Bd[mk * P:(mk + 1) * P,
                                      ig * IG * P:(ig + 1) * IG * P])
                        last = mk == nM - 1
                        for t_ in range(IG):
                            nc.tensor.matmul(psb[t_][:],
                                             bt[:, t_ * P:(t_ + 1) * P],
                                             a2[mk][:], start=(mk == 0),
                                             stop=last)
                        if gi > 0 and mk % 2 == 1:
                            x0_mm(mk // 2, GORDER[gi - 1])
                    for t_ in range(IG):
                        it = ig * IG + t_
                        nc.vector.tensor_scalar_mul(mt[it][:], psb[t_][:],
                                                    s0_t[:, it:it + 1])
                    if gi % 2 == 1:
                        # pair {GORDER[gi-1], ig} complete: in-tile pairs
                        # j = 4*GORDER[gi-1] .. +3 are ready.
                        g0 = GORDER[gi - 1]
                        build_wcombos(range(4 * g0, 4 * g0 + 4))
                for slot in range(16):
                    x0_mm(slot, GORDER[-1])
                for ob in range(nOB):
                    for h in range(2):
                        yt = ypool.tile([P, HC], F32, tag="yt")
                        nc.vector.tensor_scalar(
                            yt[:], ps0[ob][:, h * HC:(h + 1) * HC],
                            s4_t[:, ob:ob + 1], bi_t[:, ob:ob + 1], mult, add)
                        nc.sync.dma_start(
                            yTd[ob * P:(ob + 1) * P, h * HC:(h + 1) * HC],
                            yt[:])

            # phase C: Strassen chunks 1..15.
            with (
                tc.tile_pool(name="ucmb", bufs=4) as upool,
                tc.tile_pool(name="ctmp", bufs=2) as ctpool,
            ):
                def uc(k, j, i0, h0, i1, h1, op, xts, eng=None):
                    # x-combo tile = xts[i0][:, h0-half] op xts[i1][:, h1-half]
                    u = upool.tile([P, HC], F16, tag=f"u{k}", name=f"u{k}")
                    tt(u[:], xts[i0][:, h0 * HC:(h0 + 1) * HC],
                       xts[i1][:, h1 * HC:(h1 + 1) * HC], op, eng=eng)
                    return u

                def evict(t, blk, th, t0):
                    # t [128, 512] fp32 (ob2 halves side by side) ->
                    # y rows blk*128+[ob2], tokens [t0+th*256, +256)
                    for ob2 in range(2):
                        ob = blk * 2 + ob2
                        yt = ypool.tile([P, HC], F32, tag="yt")
                        nc.vector.tensor_scalar(
                            yt[:], t[:, ob2 * HC:(ob2 + 1) * HC],
                            s4_t[:, ob:ob + 1], bi_t[:, ob:ob + 1], mult, add)
                        nc.sync.dma_start(
                            yTd[ob * P:(ob + 1) * P,
                                t0 + th * HC:t0 + (th + 1) * HC], yt[:])

                for c in range(1, nTC):
                    t0 = c * TC
                    xts = []
                    for it in range(nI):
                        xt = xpool.tile([P, TC], F16, tag=f"x{it}",
                                        name=f"x{it}")
                        nc.scalar.dma_start(
                            xt[:], xTd[it * P:(it + 1) * P, t0:t0 + TC])
                        xts.append(xt)

                    # psum products [128, 512]: ob2 halves side by side.
                    pm = {}
                    for k, tg in ((1, "ps"), (2, "ps"), (3, "ps"), (4, "ps"),
                                  (6, "ps0"), (5, "ps0"), (7, "ps0")):
                        pm[k] = pspool.tile([P, TC], F32, tag=tg,
                                            name=f"pm{k}", bufs=4)

                    def prod(k, wset, mov):
                        # wset: list of 16 stationary [128, 256] tiles (or
                        # slices); mov: list of 16 moving [128, 256] APs.
                        # start=True clears the whole PSUM bank, so only the
                        # very first matmul into the tile may carry it; the
                        # second ob2 region then overwrites zeroed memory
                        # (has_written=0) which is equivalent to a start.
                        for j in range(nJ):
                            for ob2 in range(2):
                                nc.tensor.matmul(
                                    pm[k][:, ob2 * HC:(ob2 + 1) * HC],
                                    wset[j][:, ob2 * P:(ob2 + 1) * P],
                                    mov[j], start=(j == 0 and ob2 == 0),
                                    stop=(j == nJ - 1),
                                    skip_group_check=True)

                    # moving operands
                    xa = lambda i: [xts[i + j][:, 0:HC] for j in range(nJ)]
                    xb = lambda i: [xts[i + j][:, HC:TC] for j in range(nJ)]

                    # M1 = (A11+A22)(B11+B22)
                    u1 = [uc(1, j, j, 0, nJ + j, 1, add, xts)
                          for j in range(nJ)]
                    prod(1, wc[1], [u[:] for u in u1])
                    # M2 = (A21+A22) B11   (raw, strided moving operand)
                    prod(2, wc[2], xa(0))
                    # M3 = A11 (B12-B22)
                    u3 = [uc(3, j, j, 1, nJ + j, 1, sub, xts)
                          for j in range(nJ)]
                    prod(3, [mt[j][:, 0:2 * P] for j in range(nJ)],
                         [u[:] for u in u3])
                    t22 = ctpool.tile([P, TC], F32, tag="t22")
                    nc.scalar.activation(t22[:], pm[1][:], CPY)
                    tt(t22[:], t22[:], pm[2][:], sub)               # M1-M2
                    # M6 = (A21-A11)(B11+B12)
                    u6 = [uc(6, j, j, 0, j, 1, add, xts, eng=nc.gpsimd)
                          for j in range(nJ)]
                    prod(6, wc[6], [u[:] for u in u6])
                    tt(t22[:], t22[:], pm[3][:], add)               # +M3
                    tt(t22[:], t22[:], pm[6][:], add)               # +M6
                    evict(t22, 1, 1, t0)                            # C22
                    # M4 = A22 (B21-B11)
                    u4 = [uc(4, j, nJ + j, 0, j, 0, sub, xts)
                          for j in range(nJ)]
                    prod(4, [mt[nJ + j][:, 2 * P:4 * P] for j in range(nJ)],
                         [u[:] for u in u4])
                    t21 = ctpool.tile([P, TC], F32, tag="t21")
                    nc.scalar.activation(t21[:], pm[2][:], CPY)
                    tt(t21[:], t21[:], pm[4][:], add)               # M2+M4
                    evict(t21, 1, 0, t0)                            # C21
                    t11 = ctpool.tile([P, TC], F32, tag="t11")
                    nc.scalar.activation(t11[:], pm[1][:], CPY)
                    tt(t11[:], t11[:], pm[4][:], add)               # M1+M4
                    # M5 = (A11+A12) B22   (raw, strided moving operand)
                    prod(5, wc[5], xb(nJ))
                    t12 = ctpool.tile([P, TC], F32, tag="t12")
                    nc.scalar.activation(t12[:], pm[3][:], CPY)
                    tt(t12[:], t12[:], pm[5][:], add)               # M3+M5
                    evict(t12, 0, 1, t0)                            # C12
                    tt(t11[:], t11[:], pm[5][:], sub)               # -M5
                    # M7 = (A12-A22)(B21+B22)
                    u7 = [uc(7, j, nJ + j, 0, nJ + j, 1, add, xts)
                          for j in range(nJ)]
                    prod(7, wc[7], [u[:] for u in u7])
                    tt(t11[:], t11[:], pm[7][:], add)               # +M7
                    evict(t11, 0, 0, t0)                            # C11

    nc.compile()
    return nc


_NC_CACHE = None


def _get_nc():
    global _NC_CACHE
    if _NC_CACHE is None:
        _NC_CACHE = _build_nc()
    return _NC_CACHE


def _col_major(v):
    return np.ascontiguousarray(
        np.asarray(v, dtype=np.float32).reshape(-1, P).T)


def make_in_maps(x, scaling0, B, scaling2, A, scaling4, bias):
    xh = np.asarray(x, dtype=np.float32).reshape(T_ALL, IN_D).astype(np.float16)
    xT = np.ascontiguousarray(xh.T)
    B16 = np.asarray(B, dtype=np.float32).astype(np.float16)
    a2_full = (np.asarray(scaling2, dtype=np.float32)[:, None]
               * np.asarray(A, dtype=np.float32).T).astype(np.float16)
    s0c = _col_major(scaling0)

    in_maps = []
    for c in range(N_CORES):
        sh = slice(c * OS, (c + 1) * OS)
        sc = np.ascontiguousarray(np.concatenate(
            [s0c, _col_major(np.asarray(scaling4)[sh]),
             _col_major(np.asarray(bias)[sh])], axis=1))
        in_maps.append({
            "xT": xT, "B": B16,
            "a2": np.ascontiguousarray(a2_full[:, sh]),
            "sc": sc,
        })
    return in_maps


def _unshard(results):
    y = np.empty((T_ALL, OUT_D), dtype=np.float32)
    for c in range(N_CORES):
        y[:, c * OS:(c + 1) * OS] = results[c]["yT"].T
    return y.reshape(BATCH, SEQ, OUT_D)


def kernel(x, scaling0, B, scaling2, A, scaling4, bias):
    # The profile hook isn't available in every environment; force the
    # plain execution path.
    os.environ.setdefault("BASS_NEVER_TRACE", "1")

    in_maps = make_in_maps(x, scaling0, B, scaling2, A, scaling4, bias)
    nc = _get_nc()
    res = bass_utils.run_bass_kernel_spmd(
        nc, in_maps, core_ids=list(range(N_CORES)))
    return _unshard(res.results)


# revision 17
# speedup vs baseline: 1.3731x; 1.2326x over previous
"""Trainium2 Bass kernel for DoubleBinaryLinear:
    y = ((x * s0) @ B.T * s2) @ A.T * s4 + bias
with x [4, 2048, 4096] fp32 and binary (+-1) B, A [4096, 4096].

v6a: fused-weight restructure, output-sharded across the 8 cores.

    M.T = diag(s0) B.T (diag(s2) A.T)   # [in, out], token-independent
    y   = x @ M.T * s4 + bias           # x pre-cast fp16 on host

Core c computes M.T columns for its 512 output rows (one 4096x4096x512
matmul, ~220 us) and keeps M.T resident in SBUF, then streams ALL 8192
tokens through a single fused matmul (~440 us). Out-sharding makes each
core's M shard exactly what its own x-pass needs -- no collective, and
the token loop has no per-tile activations (s0 rides the M eviction
scale). All matmuls single-pass fp16 (binary weights exact; M and x
rounded once, rel err ~4e-4 << 2e-2 gate).

v6a over v5: a2 = s2*A.T is precomputed on host and shipped fp16 (kills
the on-device phase A: 32 fp8 loads + scales and their latency chain),
and the final token chunk is split in two so the tail eviction+store is
half as long.

Per-matmul moving operands must come from plain tiles: slicing a wider
(batched-DMA) tile gives the operand AP a partition stride larger than
its free extent, which drops TensorE off its fast path (measured 259 ns
vs 216 ns per 512-row matmul). So x DMAs are one trigger per tile;
evictions run on the Vector engine.
"""

import os

import numpy as np

import concourse.bacc as bacc
import concourse.mybir as mybir
from concourse import tile
from concourse import bass_utils

P = 128
F32 = mybir.dt.float32
F16 = mybir.dt.float16

IN_D = 4096
MID_D = 4096
OUT_D = 4096
BATCH = 4
SEQ = 2048
N_CORES = 8
T_ALL = BATCH * SEQ                 # 8192 tokens, every core sees all
OS = OUT_D // N_CORES               # 512 output rows per core
TC = 512                            # matmul moving free dim
nI = IN_D // P                      # 32 in tiles
nM = MID_D // P                     # 32 mid tiles
nOB = OS // P                       # 4 out blocks per core
nTC = T_ALL // TC                   # 16 token chunks
IG = 4                              # in-tiles per M-compute PSUM group

mult = mybir.AluOpType.mult
add = mybir.AluOpType.add


def _build_nc():
    nc = bacc.Bacc(None, target_bir_lowering=False)
    xTd = nc.dram_tensor("xT", [IN_D, T_ALL], F16, kind="ExternalInput")
    Bd = nc.dram_tensor("B", [MID_D, IN_D], F16, kind="ExternalInput")
    a2d = nc.dram_tensor("a2", [MID_D, OS], F16, kind="ExternalInput")
    nSC = nI + 2 * nOB
    scd = nc.dram_tensor("sc", [P, nSC], F32, kind="ExternalInput")
    yTd = nc.dram_tensor("yT", [OS, T_ALL], F32, kind="ExternalOutput")

    with tile.TileContext(nc) as tc:
        with (
            tc.tile_pool(name="consts", bufs=1) as cpool,
            tc.tile_pool(name="a2buf", bufs=1) as apool,
            tc.tile_pool(name="mtbuf", bufs=1) as mpool,
            tc.tile_pool(name="xin", bufs=2) as xpool,
            tc.tile_pool(name="bwts", bufs=32) as bpool,
            tc.tile_pool(name="yout", bufs=6) as ypool,
            tc.tile_pool(name="psum", bufs=8, space="PSUM") as pspool,
        ):
            sc_t = cpool.tile([P, nSC], F32, tag="sc")
            nc.scalar.dma_start(sc_t[:], scd[:, :])
            s0_t = sc_t[:, 0:nI]
            s4_t = sc_t[:, nI:nI + nOB]
            bi_t = sc_t[:, nI + nOB:nSC]

            # a2 = fp16(s2 * A.T) tiles [128 mid, OS], precomputed on host.
            a2 = []
            for mk in range(nM):
                a2t = apool.tile([P, OS], F16, tag=f"a{mk}", name=f"a{mk}")
                nc.scalar.dma_start(a2t[:], a2d[mk * P:(mk + 1) * P, :])
                a2.append(a2t)

            # Chunk-0 x tiles prefetch right behind the a2 loads so the
            # interleaved chunk-0 matmuls below have data early.
            x0 = []
            for it in range(nI):
                xt = xpool.tile([P, TC], F16, tag=f"x{it}", name=f"x{it}")
                nc.scalar.dma_start(xt[:], xTd[it * P:(it + 1) * P, 0:TC])
                x0.append(xt)
            # Chunk-0 accumulators held across all of phase B (4 banks);
            # phase B itself rings through the other 4.
            ps0 = [pspool.tile([P, TC], F32, tag="ps0", name="ps0", bufs=4)
                   for _ in range(nOB)]

            def x0_mm(j, ig):
                # j-th (0..15) interleave slot while phase-B group ig runs:
                # chunk-0 matmul for an in-tile of group ig-1.
                it = (ig - 1) * IG + j // nOB
                ob = j % nOB
                nc.tensor.matmul(ps0[ob][:], mt[it][:, ob * P:(ob + 1) * P],
                                 x0[it][:], start=(it == 0),
                                 stop=(it == nI - 1))

            # phase B: M.T tiles [128 in, OS]; s0 folded into eviction.
            # One chunk-0 matmul per two B-steps stretches the B-stream
            # demand timeline ~11% so DMA jitter stops stalling TensorE.
            mt = [mpool.tile([P, OS], F16, tag=f"m{it}", name=f"m{it}")
                  for it in range(nI)]
            for ig in range(nI // IG):
                psb = [pspool.tile([P, OS], F32, tag="ps", name="ps", bufs=4)
                       for _ in range(IG)]
                for mk in range(nM):
                    bt = bpool.tile([P, IG * P], F16, tag="wb")
                    nc.sync.dma_start(
                        bt[:], Bd[mk * P:(mk + 1) * P,
                                  ig * IG * P:(ig + 1) * IG * P])
                    last = mk == nM - 1
                    for t_ in range(IG):
                        nc.tensor.matmul(psb[t_][:], bt[:, t_ * P:(t_ + 1) * P],
                                         a2[mk][:], start=(mk == 0), stop=last)
                    if ig > 0 and mk % 2 == 1:
                        x0_mm(mk // 2, ig)
                for t_ in range(IG):
                    it = ig * IG + t_
                    nc.vector.tensor_scalar_mul(mt[it][:], psb[t_][:],
                                                s0_t[:, it:it + 1])
            for j in range(16):
                x0_mm(j, nI // IG)
            for ob in range(nOB):
                yt = ypool.tile([P, TC], F32, tag="yt")
                nc.vector.tensor_scalar(
                    yt[:], ps0[ob][:], s4_t[:, ob:ob + 1], bi_t[:, ob:ob + 1],
                    mult, add)
                nc.sync.dma_start(yTd[ob * P:(ob + 1) * P, 0:TC], yt[:])

            # phase C: stream remaining tokens; alternate the two 4-bank
            # PSUM rings so consecutive chunks never wait on evictions.
            # The final chunk is split in two 256-token halves so the tail
            # (eviction + store after the last matmul) is half as long.
            chunks = [(c * TC, TC) for c in range(1, nTC - 1)]
            chunks += [(15 * TC, TC // 2), (15 * TC + TC // 2, TC // 2)]
            for ci, (t0, w) in enumerate(chunks):
                xts = []
                for it in range(nI):
                    xt = xpool.tile([P, w], F16, tag=f"x{it}", name=f"x{it}")
                    nc.scalar.dma_start(
                        xt[:], xTd[it * P:(it + 1) * P, t0:t0 + w])
                    xts.append(xt)
                tag = "ps0" if ci % 2 else "ps"
                pso = [pspool.tile([P, TC], F32, tag=tag, name="pso", bufs=4)
                       for _ in range(nOB)]
                for it in range(nI):
                    for ob in range(nOB):
                        nc.tensor.matmul(pso[ob][:, 0:w],
                                         mt[it][:, ob * P:(ob + 1) * P],
                                         xts[it][:], start=(it == 0),
                                         stop=(it == nI - 1))
                for ob in range(nOB):
                    yt = ypool.tile([P, TC], F32, tag="yt")
                    nc.vector.tensor_scalar(
                        yt[:, 0:w], pso[ob][:, 0:w], s4_t[:, ob:ob + 1],
                        bi_t[:, ob:ob + 1], mult, add)
                    nc.sync.dma_start(
                        yTd[ob * P:(ob + 1) * P, t0:t0 + w], yt[:, 0:w])

    nc.compile()
    return nc


_NC_CACHE = None


def _get_nc():
    global _NC_CACHE
    if _NC_CACHE is None:
        _NC_CACHE = _build_nc()
    return _NC_CACHE


def _col_major(v):
    return np.ascontiguousarray(
        np.asarray(v, dtype=np.float32).reshape(-1, P).T)


def make_in_maps(x, scaling0, B, scaling2, A, scaling4, bias):
    xh = np.asarray(x, dtype=np.float32).reshape(T_ALL, IN_D).astype(np.float16)
    xT = np.ascontiguousarray(xh.T)
    B16 = np.asarray(B, dtype=np.float32).astype(np.float16)
    a2_full = (np.asarray(scaling2, dtype=np.float32)[:, None]
               * np.asarray(A, dtype=np.float32).T).astype(np.float16)
    s0c = _col_major(scaling0)

    in_maps = []
    for c in range(N_CORES):
        sh = slice(c * OS, (c + 1) * OS)
        sc = np.ascontiguousarray(np.concatenate(
            [s0c, _col_major(np.asarray(scaling4)[sh]),
             _col_major(np.asarray(bias)[sh])], axis=1))
        in_maps.append({
            "xT": xT, "B": B16,
            "a2": np.ascontiguousarray(a2_full[:, sh]),
            "sc": sc,
        })
    return in_maps


def _unshard(results):
    y = np.empty((T_ALL, OUT_D), dtype=np.float32)
    for c in range(N_CORES):
        y[:, c * OS:(c + 1) * OS] = results[c]["yT"].T
    return y.reshape(BATCH, SEQ, OUT_D)


def kernel(x, scaling0, B, scaling2, A, scaling4, bias):
    # The profile hook isn't available in every environment; force the
    # plain execution path.
    os.environ.setdefault("BASS_NEVER_TRACE", "1")

    in_maps = make_in_maps(x, scaling0, B, scaling2, A, scaling4, bias)
    nc = _get_nc()
    res = bass_utils.run_bass_kernel_spmd(
        nc, in_maps, core_ids=list(range(N_CORES)))
    return _unshard(res.results)


# revision 18
# speedup vs baseline: 1.3813x; 1.0059x over previous
"""Trainium2 Bass kernel for DoubleBinaryLinear:
    y = ((x * s0) @ B.T * s2) @ A.T * s4 + bias
with x [4, 2048, 4096] fp32 and binary (+-1) B, A [4096, 4096].

v6a: fused-weight restructure, output-sharded across the 8 cores.

    M.T = diag(s0) B.T (diag(s2) A.T)   # [in, out], token-independent
    y   = x @ M.T * s4 + bias           # x pre-cast fp16 on host

Core c computes M.T columns for its 512 output rows (one 4096x4096x512
matmul, ~220 us) and keeps M.T resident in SBUF, then streams ALL 8192
tokens through a single fused matmul (~440 us). Out-sharding makes each
core's M shard exactly what its own x-pass needs -- no collective, and
the token loop has no per-tile activations (s0 rides the M eviction
scale). All matmuls single-pass fp16 (binary weights exact; M and x
rounded once, rel err ~4e-4 << 2e-2 gate).

v6a over v5: a2 = s2*A.T is precomputed on host and shipped fp16 (kills
the on-device phase A: 32 fp8 loads + scales and their latency chain),
and the final token chunk is split in two so the tail eviction+store is
half as long.

Per-matmul moving operands must come from plain tiles: slicing a wider
(batched-DMA) tile gives the operand AP a partition stride larger than
its free extent, which drops TensorE off its fast path (measured 259 ns
vs 216 ns per 512-row matmul). So x DMAs are one trigger per tile;
evictions run on the Vector engine.
"""

import os

import numpy as np
import ml_dtypes

import concourse.bacc as bacc
import concourse.mybir as mybir
from concourse import tile
from concourse import bass_utils

P = 128
F32 = mybir.dt.float32
F16 = mybir.dt.float16
F8 = mybir.dt.float8e4

IN_D = 4096
MID_D = 4096
OUT_D = 4096
BATCH = 4
SEQ = 2048
N_CORES = 8
T_ALL = BATCH * SEQ                 # 8192 tokens, every core sees all
OS = OUT_D // N_CORES               # 512 output rows per core
TC = 512                            # matmul moving free dim
nI = IN_D // P                      # 32 in tiles
nM = MID_D // P                     # 32 mid tiles
nOB = OS // P                       # 4 out blocks per core
nTC = T_ALL // TC                   # 16 token chunks
IG = 4                              # in-tiles per M-compute PSUM group

mult = mybir.AluOpType.mult
add = mybir.AluOpType.add


def _build_nc():
    nc = bacc.Bacc(None, target_bir_lowering=False)
    xTd = nc.dram_tensor("xT", [IN_D, T_ALL], F16, kind="ExternalInput")
    Bd = nc.dram_tensor("B", [MID_D, IN_D], F8, kind="ExternalInput")
    a2d = nc.dram_tensor("a2", [MID_D, OS], F16, kind="ExternalInput")
    nSC = nI + 2 * nOB
    scd = nc.dram_tensor("sc", [P, nSC], F32, kind="ExternalInput")
    yTd = nc.dram_tensor("yT", [OS, T_ALL], F32, kind="ExternalOutput")

    with tile.TileContext(nc) as tc:
        with (
            tc.tile_pool(name="consts", bufs=1) as cpool,
            tc.tile_pool(name="a2buf", bufs=1) as apool,
            tc.tile_pool(name="mtbuf", bufs=1) as mpool,
            tc.tile_pool(name="xin", bufs=2) as xpool,
            tc.tile_pool(name="bwts", bufs=44) as bpool,
            tc.tile_pool(name="yout", bufs=6) as ypool,
            tc.tile_pool(name="psum", bufs=8, space="PSUM") as pspool,
        ):
            # a2 = fp16(s2 * A.T) tiles [128 mid, OS], precomputed on host.
            # These feed the very first matmuls, so they go first on the
            # scalar queue; sc is only needed at the first mt eviction.
            a2 = []
            for mk in range(nM):
                a2t = apool.tile([P, OS], F16, tag=f"a{mk}", name=f"a{mk}")
                nc.scalar.dma_start(a2t[:], a2d[mk * P:(mk + 1) * P, :])
                a2.append(a2t)

            sc_t = cpool.tile([P, nSC], F32, tag="sc")
            nc.scalar.dma_start(sc_t[:], scd[:, :])
            s0_t = sc_t[:, 0:nI]
            s4_t = sc_t[:, nI:nI + nOB]
            bi_t = sc_t[:, nI + nOB:nSC]

            # Chunk-0 x tiles prefetch right behind the a2 loads so the
            # interleaved chunk-0 matmuls below have data early.
            x0 = []
            for it in range(nI):
                xt = xpool.tile([P, TC], F16, tag=f"x{it}", name=f"x{it}")
                nc.gpsimd.dma_start(xt[:], xTd[it * P:(it + 1) * P, 0:TC])
                x0.append(xt)
            # Chunk-0 accumulators held across all of phase B (4 banks);
            # phase B itself rings through the other 4.
            ps0 = [pspool.tile([P, TC], F32, tag="ps0", name="ps0", bufs=4)
                   for _ in range(nOB)]

            def x0_mm(j, ig):
                # j-th (0..15) interleave slot while phase-B group ig runs:
                # chunk-0 matmul for an in-tile of group ig-1.
                it = (ig - 1) * IG + j // nOB
                ob = j % nOB
                nc.tensor.matmul(ps0[ob][:], mt[it][:, ob * P:(ob + 1) * P],
                                 x0[it][:], start=(it == 0),
                                 stop=(it == nI - 1))

            # phase B: M.T tiles [128 in, OS]; s0 folded into eviction.
            # One chunk-0 matmul per two B-steps stretches the B-stream
            # demand timeline ~11% so DMA jitter stops stalling TensorE.
            mt = [mpool.tile([P, OS], F16, tag=f"m{it}", name=f"m{it}")
                  for it in range(nI)]
            for ig in range(nI // IG):
                psb = [pspool.tile([P, OS], F32, tag="ps", name="ps", bufs=4)
                       for _ in range(IG)]
                for mk in range(nM):
                    bt = bpool.tile([P, IG * P], F8, tag="wb")
                    nc.sync.dma_start(
                        bt[:], Bd[mk * P:(mk + 1) * P,
                                  ig * IG * P:(ig + 1) * IG * P])
                    last = mk == nM - 1
                    for t_ in range(IG):
                        nc.tensor.matmul(psb[t_][:], bt[:, t_ * P:(t_ + 1) * P],
                                         a2[mk][:], start=(mk == 0), stop=last)
                    if ig > 0 and mk % 2 == 1:
                        x0_mm(mk // 2, ig)
                for t_ in range(IG):
                    it = ig * IG + t_
                    nc.vector.tensor_scalar_mul(mt[it][:], psb[t_][:],
                                                s0_t[:, it:it + 1])
            for j in range(16):
                x0_mm(j, nI // IG)
            for ob in range(nOB):
                yt = ypool.tile([P, TC], F32, tag="yt")
                nc.vector.tensor_scalar(
                    yt[:], ps0[ob][:], s4_t[:, ob:ob + 1], bi_t[:, ob:ob + 1],
                    mult, add)
                nc.sync.dma_start(yTd[ob * P:(ob + 1) * P, 0:TC], yt[:])

            # phase C: stream remaining tokens; alternate the two 4-bank
            # PSUM rings so consecutive chunks never wait on evictions.
            # The final chunk is split in two 256-token halves so the tail
            # (eviction + store after the last matmul) is half as long.
            chunks = [(c * TC, TC) for c in range(1, nTC - 1)]
            chunks += [(15 * TC, TC // 2), (15 * TC + TC // 2, TC // 2)]
            for ci, (t0, w) in enumerate(chunks):
                xts = []
                for it in range(nI):
                    xt = xpool.tile([P, w], F16, tag=f"x{it}", name=f"x{it}")
                    nc.scalar.dma_start(
                        xt[:], xTd[it * P:(it + 1) * P, t0:t0 + w])
                    xts.append(xt)
                tag = "ps0" if ci % 2 else "ps"
                pso = [pspool.tile([P, TC], F32, tag=tag, name="pso", bufs=4)
                       for _ in range(nOB)]
                for it in range(nI):
                    for ob in range(nOB):
                        nc.tensor.matmul(pso[ob][:, 0:w],
                                         mt[it][:, ob * P:(ob + 1) * P],
                                         xts[it][:], start=(it == 0),
                                         stop=(it == nI - 1))
                for ob in range(nOB):
                    yt = ypool.tile([P, TC], F32, tag="yt")
                    nc.vector.tensor_scalar(
                        yt[:, 0:w], pso[ob][:, 0:w], s4_t[:, ob:ob + 1],
                        bi_t[:, ob:ob + 1], mult, add)
                    nc.sync.dma_start(
                        yTd[ob * P:(ob + 1) * P, t0:t0 + w], yt[:, 0:w])

    nc.compile()
    return nc


_NC_CACHE = None


def _get_nc():
    global _NC_CACHE
    if _NC_CACHE is None:
        _NC_CACHE = _build_nc()
    return _NC_CACHE


def _col_major(v):
    return np.ascontiguousarray(
        np.asarray(v, dtype=np.float32).reshape(-1, P).T)


def make_in_maps(x, scaling0, B, scaling2, A, scaling4, bias):
    xh = np.asarray(x, dtype=np.float32).reshape(T_ALL, IN_D).astype(np.float16)
    xT = np.ascontiguousarray(xh.T)
    B8 = np.asarray(B, dtype=np.float32).astype(ml_dtypes.float8_e4m3)
    a2_full = (np.asarray(scaling2, dtype=np.float32)[:, None]
               * np.asarray(A, dtype=np.float32).T).astype(np.float16)
    s0c = _col_major(scaling0)

    in_maps = []
    for c in range(N_CORES):
        sh = slice(c * OS, (c + 1) * OS)
        sc = np.ascontiguousarray(np.concatenate(
            [s0c, _col_major(np.asarray(scaling4)[sh]),
             _col_major(np.asarray(bias)[sh])], axis=1))
        in_maps.append({
            "xT": xT, "B": B8,
            "a2": np.ascontiguousarray(a2_full[:, sh]),
            "sc": sc,
        })
    return in_maps


def _unshard(results):
    y = np.empty((T_ALL, OUT_D), dtype=np.float32)
    for c in range(N_CORES):
        y[:, c * OS:(c + 1) * OS] = results[c]["yT"].T
    return y.reshape(BATCH, SEQ, OUT_D)


def kernel(x, scaling0, B, scaling2, A, scaling4, bias):
    # The profile hook isn't available in every environment; force the
    # plain execution path.
    os.environ.setdefault("BASS_NEVER_TRACE", "1")

    in_maps = make_in_maps(x, scaling0, B, scaling2, A, scaling4, bias)
    nc = _get_nc()
    res = bass_utils.run_bass_kernel_spmd(
        nc, in_maps, core_ids=list(range(N_CORES)))
    return _unshard(res.results)
